# revision 23
# baseline (speedup 1.0000x reference)
"""Trainium2 Bass kernel for nn_DvoAm_EncTrackRefining (8-core SPMD).

Strategy (tensor parallel over channels, per sharding hint):
  - Each of the 8 cores owns a 128-channel slice `ck = [128k, 128k+128)`:
    conv1 / conv2 output channels, ConvLSTM gate channels (i, o, g -- the
    forget gate is dead because c0 = 0, and the h0 half of the lstm conv
    input is dead because h0 = 0), and the attention memory bank slice
    memory[:, :, ck].
  - Host-side prep: weights pre-transposed to contraction-major [ci, tap, co]
    layout and (optionally) cast to fp16; activations pre-padded spatially so
    3x3 SAME conv taps become strided access patterns over padded tiles.
  - Cross-core collectives: one small stats AllReduce (attention cosine
    partial sums over channels), three AllGathers (M_dash, y1, xA padded
    channel blocks), one final 7x4 pose AllReduce.

The kernel is self-contained: `kernel(**inputs)` takes full unsharded inputs
as in reference.setup_inputs() and returns the full [4, 7] output.
"""

import os
import sys

import numpy as np

for _p in ("/opt/trn_rl_repo",):
    if os.path.isdir(_p) and _p not in sys.path:
        sys.path.insert(0, _p)

# ---------------------------------------------------------------- constants
B, C, H, W, N = 4, 1024, 8, 10, 11
HW = H * W                    # 80
BHW = B * HW                  # 320
Hp, Wp = H + 2, W + 2         # 10, 12
PAD = B * Hp * Wp             # 480
P = 128                       # partitions = per-core channel slice
NCORES = 8
CT1 = (2 * C) // P            # 16 ci tiles for conv1 (enc + mdash)
CT2 = C // P                  # 8 ci tiles for conv2 / lstm
NG = 3                        # live lstm gates: i, o, g  (f is dead)
EPS = 1e-8
WCHUNK = 4                    # ci tiles per weight DMA chunk

# matmul dtype for the conv path ("float16" or "float32")
MM_DT = os.environ.get("KERNEL_MM_DT", "float16")

_cache = {}


# ---------------------------------------------------------------- program
def build_program(mm_dt_name=MM_DT):
    import concourse.bass as bass
    import concourse.tile as tile
    from concourse import bacc, mybir
    from contextlib import ExitStack

    f32 = mybir.dt.float32
    mdt = getattr(mybir.dt, mm_dt_name)
    AF = mybir.ActivationFunctionType
    ALU = mybir.AluOpType
    AX = mybir.AxisListType

    nc = bacc.Bacc("TRN2", target_bir_lowering=False, debug=False,
                   num_devices=NCORES)
    RG = [list(range(NCORES))]
    HB = B // 2                   # batch half
    PADH = HB * Hp * Wp           # 240: padded elems per batch half

    # ---- external I/O (per-core shapes; contents differ per core) ----
    mem = nc.dram_tensor("mem", [P, N, B, HW], f32, kind="ExternalInput")
    oap = nc.dram_tensor("oap", [P, B, HW], f32, kind="ExternalInput")
    htr = nc.dram_tensor("htr", [P, B, HW], f32, kind="ExternalInput")
    xenc = nc.dram_tensor("xenc", [P, CT2, PAD], mdt, kind="ExternalInput")
    w1 = nc.dram_tensor("w1", [P, CT1, 9, P], mdt, kind="ExternalInput")
    w2 = nc.dram_tensor("w2", [P, CT2, 9, P], mdt, kind="ExternalInput")
    w4 = nc.dram_tensor("w4", [P, NG, CT2, 9, P], mdt, kind="ExternalInput")
    b1 = nc.dram_tensor("b1", [P, 1], f32, kind="ExternalInput")
    b2 = nc.dram_tensor("b2", [P, 1], f32, kind="ExternalInput")
    b4 = nc.dram_tensor("b4", [P, NG], f32, kind="ExternalInput")
    fct = nc.dram_tensor("fct", [P, 7], f32, kind="ExternalInput")
    fcb = nc.dram_tensor("fcb", [7, 1], f32, kind="ExternalInput")
    pose = nc.dram_tensor("pose", [7, B], f32, kind="ExternalOutput")

    with tile.TileContext(nc) as tc, ExitStack() as ctx:
        dram = ctx.enter_context(tc.tile_pool(name="dram", bufs=1, space="DRAM"))
        const = ctx.enter_context(tc.tile_pool(name="const", bufs=1))
        att = ctx.enter_context(tc.tile_pool(name="att", bufs=1))
        prods = ctx.enter_context(tc.tile_pool(name="prods", bufs=4))
        small = ctx.enter_context(tc.tile_pool(name="small", bufs=1))
        wpool = ctx.enter_context(tc.tile_pool(name="wpool", bufs=5))
        xpool = ctx.enter_context(tc.tile_pool(name="xpool", bufs=1))
        pconv = ctx.enter_context(tc.tile_pool(name="pconv", bufs=3, space="PSUM"))
        plstm = ctx.enter_context(tc.tile_pool(name="plstm", bufs=1, space="PSUM"))
        psml = ctx.enter_context(tc.tile_pool(name="psml", bufs=2, space="PSUM"))

        # ---- DRAM scratch ----
        stats_loc = dram.tile([24, BHW], f32)
        stats_sh = dram.tile([24, BHW], f32, addr_space="Shared")
        adram = dram.tile([12, BHW], mdt)
        md_loc = [dram.tile([P, PADH], mdt, name=f"md_loc{h}") for h in range(2)]
        md_sh = [dram.tile([NCORES, P, PADH], mdt, addr_space="Shared",
                           name=f"md_sh{h}") for h in range(2)]
        y1_loc = [dram.tile([P, PADH], mdt, name=f"y1_loc{h}") for h in range(2)]
        y1_sh = [dram.tile([NCORES, P, PADH], mdt, addr_space="Shared",
                           name=f"y1_sh{h}") for h in range(2)]
        xa_loc = [dram.tile([P, PADH], mdt, name=f"xa_loc{h}") for h in range(2)]
        xa_sh = [dram.tile([NCORES, P, PADH], mdt, addr_space="Shared",
                           name=f"xa_sh{h}") for h in range(2)]

        # ---- warmup collective: pulls the ncfw init barrier to t~0 so it
        #      overlaps the attention prologue ----
        warm_loc = dram.tile([1, 8], f32)
        warm_sh = dram.tile([1, 8], f32, addr_space="Shared")
        warm_s = small.tile([1, 8], f32, name="warm_s")
        nc.vector.memset(warm_s[:], 0.0)
        nc.gpsimd.dma_start(warm_loc[:], warm_s[:])
        nc.gpsimd.collective_compute(
            "AllGather", mybir.AluOpType.bypass, replica_groups=RG,
            ins=[warm_loc[0:1, 0:1].opt()], outs=[warm_sh[:].opt()])

        # ---- constants / small loads ----
        b1_s = const.tile([P, 1], f32)
        nc.sync.dma_start(b1_s[:], b1[:])
        b2_s = const.tile([P, 1], f32)
        nc.sync.dma_start(b2_s[:], b2[:])
        b4_s = const.tile([P, NG], f32)
        nc.sync.dma_start(b4_s[:], b4[:])
        fct_s = const.tile([P, 7], f32)
        nc.sync.dma_start(fct_s[:], fct[:])
        fcb_s = const.tile([7, 1], f32)
        nc.sync.dma_start(fcb_s[:], fcb[:])
        # one-hot column bank: Z[:, 23] = 1, else 0.  view(i) = Z[:, 23-i:47-i]
        # is a [P, 24] matrix whose column i is all-ones.
        Z = const.tile([P, 47], f32)
        nc.vector.memset(Z[:], 0.0)
        nc.vector.memset(Z[:, 23:24], 1.0)

        def ones_lhsT(i, m=24, kdim=P):
            return Z[0:kdim, 23 - i:23 - i + m]

        # ---- attention inputs + encoder half of conv1 input ----
        mem_s = att.tile([P, N, B, HW], f32)
        nc.sync.dma_start(mem_s[:], mem[:])
        oap_s = att.tile([P, B, HW], f32)
        nc.sync.dma_start(oap_s[:], oap[:])
        htr_s = att.tile([P, B, HW], f32)
        nc.sync.dma_start(htr_s[:], htr[:])
        x4e = xpool.tile([P, CT2, B, Hp, Wp], mdt, name="x4e")
        nc.sync.dma_start(x4e[:].rearrange("p t b h w -> p t (b h w)"), xenc[:])

        # ---- local attention stats (partial over this core's 128 channels) --
        # stats-AllReduce-critical products first (PE consumes them asap)
        prA = att.tile([P, N, B, HW], f32, tag="bigA")
        nc.vector.tensor_mul(prA[:], mem_s[:],
                             oap_s[:, None].broadcast_to([P, N, B, HW]))
        mem2 = att.tile([P, N, B, HW], f32)
        nc.vector.tensor_mul(mem2[:], mem_s[:], mem_s[:])
        oap2 = prods.tile([P, B, HW], f32, name="oap2", tag="pb")
        nc.vector.tensor_mul(oap2[:], oap_s[:], oap_s[:])
        prB = att.tile([P, N, B, HW], f32, tag="bigB")
        nc.vector.tensor_mul(prB[:], mem_s[:],
                             htr_s[:, None].broadcast_to([P, N, B, HW]))
        numB = att.tile([P, N, B], f32)
        nc.vector.reduce_sum(numB[:], prB[:], axis=AX.X)
        htr2 = att.tile([P, B, HW], f32)
        nc.vector.tensor_mul(htr2[:], htr_s[:], htr_s[:])
        htr2s = att.tile([P, B], f32)
        nc.vector.reduce_sum(htr2s[:], htr2[:], axis=AX.X)
        mem2s = att.tile([P, N, B], f32)
        nc.vector.reduce_sum(mem2s[:], mem2[:], axis=AX.X)
        sqm = att.tile([P, N, B], f32)
        nc.scalar.activation(sqm[:], mem2s[:], AF.Sqrt)
        sqh = att.tile([P, B], f32)
        nc.scalar.activation(sqh[:], htr2s[:], AF.Sqrt)
        denb = att.tile([P, N, B], f32)
        nc.vector.tensor_mul(denb[:], sqm[:],
                             sqh[:, None, :].broadcast_to([P, N, B]))
        nc.vector.tensor_scalar_max(denb[:], denb[:], EPS)
        rdb = att.tile([P, N, B], f32)
        nc.vector.reciprocal_approx_fast(rdb[:], denb[:])
        csb = att.tile([P, N, B], f32)
        nc.vector.tensor_mul(csb[:], numB[:], rdb[:])

        # partition-sum stats into PSUM via one-hot matmuls:
        # rows 0..10 num_a[n]; 11..21 |mem_n|^2; 22 |oap|^2; 23 zero
        stats_ps = psml.tile([24, BHW], f32, name="stats_ps", tag="sm")
        nmm = 2 * N + 1
        k = 0
        for n in range(N):
            nc.tensor.matmul(stats_ps[:], lhsT=ones_lhsT(n),
                             rhs=prA[:, n].rearrange("p b x -> p (b x)"),
                             start=(k == 0), stop=(k == nmm - 1))
            k += 1
        for n in range(N):
            nc.tensor.matmul(stats_ps[:], lhsT=ones_lhsT(11 + n),
                             rhs=mem2[:, n].rearrange("p b x -> p (b x)"),
                             start=False, stop=(k == nmm - 1))
            k += 1
        nc.tensor.matmul(stats_ps[:], lhsT=ones_lhsT(22),
                         rhs=oap2[:].rearrange("p b x -> p (b x)"),
                         start=False, stop=True)
        csb_ps = psml.tile([1, N * B], f32, name="csb_ps", tag="sm")
        nc.tensor.matmul(csb_ps[:], lhsT=Z[:, 23:24],
                         rhs=csb[:].rearrange("p n b -> p (n b)"))

        # pack + AllReduce
        staging = small.tile([24, BHW], f32, name="staging")
        nc.vector.tensor_copy(staging[:], stats_ps[:])
        csb_s = small.tile([1, BHW], f32, name="csb_s")
        nc.vector.memset(csb_s[:], 0.0)
        nc.vector.tensor_copy(csb_s[0:1, 0:N * B], csb_ps[:])
        nc.scalar.dma_start(stats_loc[0:23, :], staging[0:23, :])
        nc.scalar.dma_start(stats_loc[23:24, :], csb_s[:])
        nc.gpsimd.collective_compute(
            "AllReduce", mybir.AluOpType.add, replica_groups=RG,
            ins=[stats_loc[:].opt()], outs=[stats_sh[:].opt()])

        # pre-AllReduce shadow work: w_n = mem_n * csb_n in place over mem_s
        nc.vector.tensor_mul(
            mem_s[:], mem_s[:],
            csb[:, :, :, None].broadcast_to([P, N, B, HW]))

        def conv_part(ps, wdram, wsel, xtiles, trange, tag, start, stop,
                      bh=None, skip_gc=False):
            """Accumulate 3x3 conv ci-tiles over trange into psum ps.
            bh: None = all batches; 0/1 = batch half."""
            tlist = list(trange)
            first, last = True, False
            for c0 in range(0, len(tlist), WCHUNK):
                chunk = tlist[c0:c0 + WCHUNK]
                cn = len(chunk)
                wc = wpool.tile([P, cn, 9, P], mdt, name=f"wc_{tag}_{c0}",
                                tag="w")
                nc.sync.dma_start(wc[:], wsel(wdram, chunk[0], cn))
                for ti, t in enumerate(chunk):
                    xt = xtiles(t)
                    if bh is None:
                        xv = xt[:, :, :, :]
                    else:
                        xv = xt[:, bh * HB:(bh + 1) * HB]
                    for j in range(9):
                        dy, dx = j // 3, j % 3
                        last = (c0 + ti == len(tlist) - 1) and (j == 8)
                        nc.tensor.matmul(
                            ps[:], lhsT=wc[:, ti, j, :],
                            rhs=xv[:, :, dy:dy + H, dx:dx + W],
                            start=(start and first), stop=(stop and last),
                            skip_group_check=skip_gc)
                        first = False

        # ---- conv1 encoder half (full batch): runs during barrier + AR.
        #      The group is left open (stop=False); the memory-half matmuls
        #      accumulate into per-batch-half subregions of the same tile ----
        y1p = pconv.tile([P, B, H, W], f32, name="y1p", tag="cv")
        conv_part(y1p, w1, lambda wd, t0, cn: wd[:, t0:t0 + cn],
                  lambda t: x4e[:, t], range(CT2), "w1a", True, False)

        # ---- post-AllReduce: g_n = e_n * rcb_n, rs ----
        na_g = small.tile([N, BHW], f32, name="na_g")
        nc.scalar.dma_start(na_g[:], stats_sh[0:N, :])
        ms_g = small.tile([N, BHW], f32, name="ms_g")
        nc.sync.dma_start(ms_g[:], stats_sh[N:2 * N, :])
        ob_g = small.tile([N, BHW], f32, name="ob_g")
        nc.scalar.dma_start(ob_g[:], stats_sh[22, :][None].broadcast_to([N, BHW]))
        cbs = small.tile([N, B], f32, name="cbs")
        nc.sync.dma_start(cbs[:], stats_sh[23, 0:N * B].rearrange(
            "(n b) -> n b", n=N))
        sqa = small.tile([N, BHW], f32, name="sqa")
        nc.scalar.activation(sqa[:], ms_g[:], AF.Sqrt)
        sqo = small.tile([N, BHW], f32, name="sqo")
        nc.scalar.activation(sqo[:], ob_g[:], AF.Sqrt)
        dena = small.tile([N, BHW], f32, name="dena")
        nc.vector.tensor_mul(dena[:], sqa[:], sqo[:])
        nc.vector.tensor_scalar_max(dena[:], dena[:], EPS)
        rda = small.tile([N, BHW], f32, name="rda")
        nc.vector.reciprocal_approx_fast(rda[:], dena[:])
        estage = small.tile([N, BHW], f32, name="estage")
        csa = small.tile([N, BHW], f32, name="csa")
        nc.vector.tensor_mul(csa[:], na_g[:], rda[:])
        nc.scalar.activation(estage[:], csa[:], AF.Exp)
        se_ps = psml.tile([1, BHW], f32, name="se_ps", tag="sm")
        nc.tensor.matmul(se_ps[:], lhsT=Z[0:N, 23:24], rhs=estage[:])
        rs_s = small.tile([1, BHW], f32, name="rs_s")
        sef = small.tile([1, BHW], f32, name="sef")
        nc.vector.tensor_copy(sef[:], se_ps[:])
        nc.vector.reciprocal_approx_fast(rs_s[:], sef[:])
        rcbs = small.tile([N, B], f32, name="rcbs")
        nc.vector.tensor_scalar_add(cbs[:], cbs[:], EPS)
        nc.vector.reciprocal_approx_fast(rcbs[:], cbs[:])
        gst = small.tile([N, B, HW], mdt, name="gst")
        nc.vector.tensor_mul(gst[:],
                             estage[:].rearrange("n (b x) -> n b x", b=B),
                             rcbs[:, :, None].broadcast_to([N, B, HW]))
        rs16 = small.tile([1, BHW], mdt, name="rs16")
        nc.vector.tensor_copy(rs16[:], rs_s[:])
        nc.scalar.dma_start(adram[0:N, :], gst[:].rearrange("n b x -> n (b x)"))
        nc.scalar.dma_start(adram[N:N + 1, :], rs16[:])
        ab = att.tile([P, 12, B, HW], mdt)
        nc.scalar.dma_start(ab[:].rearrange("p r b x -> p r (b x)"),
                            adram[:][None].broadcast_to([P, 12, BHW]))

        # ---- M_dash halves: macc_h = rs * sum_n g_n * w_n,
        #      each half AllGathers while the other computes ----
        x4m = xpool.tile([P, CT2, B, Hp, Wp], mdt, name="x4m")
        for h in range(2):
            bs = slice(h * HB, (h + 1) * HB)
            gw = att.tile([P, N, HB, HW], f32, name=f"gw{h}", tag="bigB")
            nc.vector.tensor_mul(gw[:], mem_s[:, :, bs], ab[:, 0:N, bs])
            macc = prods.tile([P, HB, HW], f32, name=f"macc{h}", tag="pb")
            nc.vector.reduce_sum(
                macc[:], gw[:].rearrange("p n b x -> p b x n"), axis=AX.X)
            mpad = xpool.tile([P, HB, Hp, Wp], mdt, name=f"mpad{h}")
            nc.vector.memset(mpad[:], 0.0)
            for b in range(HB):
                nc.vector.tensor_mul(
                    mpad[:, b, 1:1 + H, 1:1 + W],
                    macc[:, b].rearrange("p (h w) -> p h w", h=H),
                    ab[:, N, h * HB + b].rearrange("p (h w) -> p h w", h=H))
            nc.scalar.dma_start(md_loc[h][:],
                                mpad[:].rearrange("p b h w -> p (b h w)"))
            nc.gpsimd.collective_compute(
                "AllGather", mybir.AluOpType.bypass, replica_groups=RG,
                ins=[md_loc[h][:].opt()], outs=[md_sh[h][:].opt()])
            for t in range(CT2):
                nc.sync.dma_start(
                    x4m[:, t, bs].rearrange("p b h w -> p (b h w)"),
                    md_sh[h][t])

        # ---- conv1 memory half, per batch half; epilogue + y1 AllGather ----
        x4y = xpool.tile([P, CT2, B, Hp, Wp], mdt, name="x4y")
        for h in range(2):
            bs = slice(h * HB, (h + 1) * HB)
            yv = y1p[:, bs]
            conv_part(yv, w1, lambda wd, t0, cn: wd[:, t0:t0 + cn],
                      lambda t: x4m[:, t - CT2], range(CT2, CT1), f"w1b{h}",
                      False, True, bh=h, skip_gc=(h == 1))
            yb = small.tile([P, HB, H, W], f32, name=f"yb{h}")
            nc.vector.tensor_scalar_add(yb[:], yv, b1_s[:])
            y1pad = xpool.tile([P, HB, Hp, Wp], mdt, name=f"y1pad{h}")
            nc.vector.memset(y1pad[:], 0.0)
            for b in range(HB):
                nc.vector.scalar_tensor_tensor(
                    y1pad[:, b, 1:1 + H, 1:1 + W], yb[:, b], 0.1, yb[:, b],
                    op0=ALU.mult, op1=ALU.max)
            nc.scalar.dma_start(y1_loc[h][:],
                                y1pad[:].rearrange("p b h w -> p (b h w)"))
            nc.gpsimd.collective_compute(
                "AllGather", mybir.AluOpType.bypass, replica_groups=RG,
                ins=[y1_loc[h][:].opt()], outs=[y1_sh[h][:].opt()])
            for t in range(CT2):
                nc.sync.dma_start(
                    x4y[:, t, bs].rearrange("p b h w -> p (b h w)"),
                    y1_sh[h][t])

        # ---- conv2 per batch half; epilogue + xA AllGather ----
        x4x = xpool.tile([P, CT2, B, Hp, Wp], mdt, name="x4x")
        for h in range(2):
            bs = slice(h * HB, (h + 1) * HB)
            xq = pconv.tile([P, HB, H, W], f32, name=f"xq{h}", tag="cv")
            conv_part(xq, w2, lambda wd, t0, cn: wd[:, t0:t0 + cn],
                      lambda t: x4y[:, t], range(CT2), f"w2{h}",
                      True, True, bh=h)
            xb = small.tile([P, HB, H, W], f32, name=f"xb{h}")
            nc.vector.tensor_scalar_add(xb[:], xq[:], b2_s[:])
            xapad = xpool.tile([P, HB, Hp, Wp], mdt, name=f"xapad{h}")
            nc.vector.memset(xapad[:], 0.0)
            for b in range(HB):
                nc.vector.scalar_tensor_tensor(
                    xapad[:, b, 1:1 + H, 1:1 + W], xb[:, b], 0.1, xb[:, b],
                    op0=ALU.mult, op1=ALU.max)
            nc.scalar.dma_start(xa_loc[h][:],
                                xapad[:].rearrange("p b h w -> p (b h w)"))
            nc.gpsimd.collective_compute(
                "AllGather", mybir.AluOpType.bypass, replica_groups=RG,
                ins=[xa_loc[h][:].opt()], outs=[xa_sh[h][:].opt()])
            for t in range(CT2):
                nc.sync.dma_start(
                    x4x[:, t, bs].rearrange("p b h w -> p (b h w)"),
                    xa_sh[h][t])

        # ---- ConvLSTM gates, full batch.  Order i, g, o so the c-path
        #      nonlinearities overlap the o-gate matmuls ----
        gps = {}
        for g in (0, 2, 1):
            gp = plstm.tile([P, B, H, W], f32, name=f"gate{g}", tag=f"g{g}")
            conv_part(gp, w4, lambda wd, t0, cn, g=g: wd[:, g, t0:t0 + cn],
                      lambda t: x4x[:, t], range(CT2), f"w4g{g}", True, True)
            gps[g] = gp
            if g == 0:
                si = small.tile([P, B, H, W], f32, name="si")
                nc.scalar.activation(si[:], gp[:], AF.Sigmoid,
                                     bias=b4_s[:, 0:1])
            elif g == 2:
                tg = small.tile([P, B, H, W], f32, name="tg")
                nc.scalar.activation(tg[:], gp[:], AF.Tanh, bias=b4_s[:, 2:3])
                cc = small.tile([P, B, H, W], f32, name="cc")
                nc.vector.tensor_mul(cc[:], si[:], tg[:])
                th = small.tile([P, B, H, W], f32, name="th")
                nc.scalar.activation(th[:], cc[:], AF.Tanh)
        so = small.tile([P, B, H, W], f32, name="so")
        nc.scalar.activation(so[:], gps[1][:], AF.Sigmoid, bias=b4_s[:, 1:2])
        hh = small.tile([P, B, H, W], f32, name="hh")
        nc.vector.tensor_mul(hh[:], so[:], th[:])

        # ---- gap + fc (1/HW folded into fct; b_fc/NCORES folded into fcb) ----
        gap = small.tile([P, B], f32, name="gap")
        nc.vector.reduce_sum(gap[:], hh[:], axis=AX.XY)
        pose_ps = psml.tile([7, B], f32, name="pose_ps", tag="sm")
        nc.tensor.matmul(pose_ps[:], lhsT=fct_s[:], rhs=gap[:])
        posec = small.tile([7, B], f32, name="posec")
        nc.scalar.activation(posec[:], pose_ps[:], AF.Identity, bias=fcb_s[:])
        # per-core partial pose (b_fc/NCORES folded in); host gather sums the
        # 8 sum-shards into the full output
        nc.scalar.dma_start(pose[:], posec[:])

    nc.compile()
    return nc


# ---------------------------------------------------------------- host prep
def prep_core_inputs(inputs, mm_dt_name=MM_DT):
    """Build the 8 per-core input maps from the full problem inputs."""
    mm_np = np.float16 if mm_dt_name == "float16" else np.float32
    f32 = np.float32

    memory = np.asarray(inputs["memory"], f32)      # [N,B,C,H,W]
    out_enc = np.asarray(inputs["out_enc"], f32)    # [B,C,H,W]
    h_track = np.asarray(inputs["h_track"], f32)
    outA_prev = np.asarray(inputs["outA_prev"], f32)
    w_conv1 = np.asarray(inputs["w_conv1"], f32)    # [C,2C,3,3]
    b_conv1 = np.asarray(inputs["b_conv1"], f32)
    w_conv2 = np.asarray(inputs["w_conv2"], f32)    # [C,C,3,3]
    b_conv2 = np.asarray(inputs["b_conv2"], f32)
    w_lstm = np.asarray(inputs["w_lstm"], f32)      # [4C,2C,3,3]
    b_lstm = np.asarray(inputs["b_lstm"], f32)
    w_fc = np.asarray(inputs["w_fc"], f32)          # [7,C]
    b_fc = np.asarray(inputs["b_fc"], f32)

    # shared padded encoder activations: [P, CT2, B*Hp*Wp]
    xe = np.zeros((C, B, Hp, Wp), f32)
    xe[:, :, 1:1 + H, 1:1 + W] = out_enc.transpose(1, 0, 2, 3)
    xenc = np.ascontiguousarray(
        xe.reshape(CT2, P, B * Hp * Wp).transpose(1, 0, 2)).astype(mm_np)

    def wt(wslice, nt):
        # [128co, nt*128ci, 3, 3] -> [128ci, nt, 9, 128co]
        a = wslice.reshape(P, nt, P, 9)
        return np.ascontiguousarray(a.transpose(2, 1, 3, 0)).astype(mm_np)

    gbase = [0, 2 * C, 3 * C]   # i, o, g rows in w_lstm / b_lstm

    maps = []
    for k in range(NCORES):
        ck = slice(k * P, (k + 1) * P)
        m = {}
        m["mem"] = np.ascontiguousarray(
            memory[:, :, ck].transpose(2, 0, 1, 3, 4).reshape(P, N, B, HW))
        m["oap"] = np.ascontiguousarray(
            outA_prev[:, ck].transpose(1, 0, 2, 3).reshape(P, B, HW))
        m["htr"] = np.ascontiguousarray(
            h_track[:, ck].transpose(1, 0, 2, 3).reshape(P, B, HW))
        m["xenc"] = xenc
        m["w1"] = wt(w_conv1[ck].reshape(P, 2 * C, 9), CT1)
        m["w2"] = wt(w_conv2[ck].reshape(P, C, 9), CT2)
        m["w4"] = np.ascontiguousarray(np.stack(
            [wt(w_lstm[g + k * P:g + (k + 1) * P, :C].reshape(P, C, 9), CT2)
             for g in gbase], axis=1))
        m["b1"] = b_conv1[ck].reshape(P, 1).copy()
        m["b2"] = b_conv2[ck].reshape(P, 1).copy()
        m["b4"] = np.ascontiguousarray(
            np.stack([b_lstm[g + k * P:g + (k + 1) * P] for g in gbase], axis=1))
        m["fct"] = np.ascontiguousarray(w_fc[:, ck].T) / float(HW)
        m["fcb"] = (b_fc / float(NCORES)).reshape(7, 1).copy()
        maps.append(m)
    return maps


# ---------------------------------------------------------------- entry
def run(inputs, trace=False, mm_dt_name=MM_DT):
    from concourse.bass_utils import run_bass_kernel_spmd

    key = ("prog", mm_dt_name)
    if key not in _cache:
        _cache[key] = build_program(mm_dt_name)
    nc = _cache[key]
    in_maps = prep_core_inputs(inputs, mm_dt_name)
    res = run_bass_kernel_spmd(nc, in_maps, list(range(NCORES)), trace=trace)
    acc = np.zeros((7, B), np.float64)
    for k in range(NCORES):
        acc += np.asarray(res.results[k]["pose"], np.float32)
    out = acc.T.astype(np.float32)  # [B, 7]
    return out, res


def kernel(**inputs) -> np.ndarray:
    out, _ = run(inputs, trace=False)
    return out


# revision 24
# speedup vs baseline: 1.5035x; 1.5035x over previous
"""Trainium2 Bass kernel for nn_DvoAm_EncTrackRefining (8-core SPMD).

Strategy (tensor parallel over channels, per sharding hint):
  - Each of the 8 cores owns a 128-channel slice `ck = [128k, 128k+128)`:
    conv1 / conv2 output channels, ConvLSTM gate channels (i, o, g -- the
    forget gate is dead because c0 = 0, and the h0 half of the lstm conv
    input is dead because h0 = 0), and the attention memory bank slice
    memory[:, :, ck].
  - Host-side prep: weights pre-transposed to contraction-major [ci, tap, co]
    layout and (optionally) cast to fp16; activations pre-padded spatially so
    3x3 SAME conv taps become strided access patterns over padded tiles.
  - Cross-core collectives: one small stats AllReduce (attention cosine
    partial sums over channels), three AllGathers (M_dash, y1, xA padded
    channel blocks), one final 7x4 pose AllReduce.

The kernel is self-contained: `kernel(**inputs)` takes full unsharded inputs
as in reference.setup_inputs() and returns the full [4, 7] output.
"""

import os
import sys

import numpy as np

for _p in ("/opt/trn_rl_repo",):
    if os.path.isdir(_p) and _p not in sys.path:
        sys.path.insert(0, _p)

# ---------------------------------------------------------------- constants
B, C, H, W, N = 4, 1024, 8, 10, 11
HW = H * W                    # 80
BHW = B * HW                  # 320
Hp, Wp = H + 2, W + 2         # 10, 12
PAD = B * Hp * Wp             # 480
P = 128                       # partitions = per-core channel slice
NCORES = 8
CT1 = (2 * C) // P            # 16 ci tiles for conv1 (enc + mdash)
CT2 = C // P                  # 8 ci tiles for conv2 / lstm
NG = 3                        # live lstm gates: i, o, g  (f is dead)
EPS = 1e-8
WCHUNK = 4                    # ci tiles per weight DMA chunk

# matmul dtype for the conv path ("float16" or "float32")
MM_DT = os.environ.get("KERNEL_MM_DT", "float16")

_cache = {}


# ---------------------------------------------------------------- program
def build_program(mm_dt_name=MM_DT):
    import concourse.bass as bass
    import concourse.tile as tile
    from concourse import bacc, mybir
    from contextlib import ExitStack

    f32 = mybir.dt.float32
    mdt = getattr(mybir.dt, mm_dt_name)
    AF = mybir.ActivationFunctionType
    ALU = mybir.AluOpType
    AX = mybir.AxisListType

    nc = bacc.Bacc("TRN2", target_bir_lowering=False, debug=False,
                   num_devices=NCORES)
    RG = [list(range(NCORES))]
    HB = B // 2                   # batch half
    PADH = HB * Hp * Wp           # 240: padded elems per batch half

    # ---- external I/O (per-core shapes; contents differ per core) ----
    mem = nc.dram_tensor("mem", [P, N, B, HW], f32, kind="ExternalInput")
    oap = nc.dram_tensor("oap", [P, B, HW], f32, kind="ExternalInput")
    htr = nc.dram_tensor("htr", [P, B, HW], f32, kind="ExternalInput")
    xenc = nc.dram_tensor("xenc", [P, CT2, PAD], mdt, kind="ExternalInput")
    w1 = nc.dram_tensor("w1", [P, CT1, 9, P], mdt, kind="ExternalInput")
    w2 = nc.dram_tensor("w2", [P, CT2, 9, P], mdt, kind="ExternalInput")
    w4 = nc.dram_tensor("w4", [P, NG, CT2, 9, P], mdt, kind="ExternalInput")
    b1 = nc.dram_tensor("b1", [P, 1], f32, kind="ExternalInput")
    b2 = nc.dram_tensor("b2", [P, 1], f32, kind="ExternalInput")
    b4 = nc.dram_tensor("b4", [P, NG], f32, kind="ExternalInput")
    fct = nc.dram_tensor("fct", [P, 7], f32, kind="ExternalInput")
    fcb = nc.dram_tensor("fcb", [7, 1], f32, kind="ExternalInput")
    pose = nc.dram_tensor("pose", [7, B], f32, kind="ExternalOutput")

    with tile.TileContext(nc) as tc, ExitStack() as ctx:
        dram = ctx.enter_context(tc.tile_pool(name="dram", bufs=1, space="DRAM"))
        const = ctx.enter_context(tc.tile_pool(name="const", bufs=1))
        att = ctx.enter_context(tc.tile_pool(name="att", bufs=1))
        prods = ctx.enter_context(tc.tile_pool(name="prods", bufs=4))
        small = ctx.enter_context(tc.tile_pool(name="small", bufs=1))
        wpool = ctx.enter_context(tc.tile_pool(name="wpool", bufs=5))
        xpool = ctx.enter_context(tc.tile_pool(name="xpool", bufs=1))
        pconv = ctx.enter_context(tc.tile_pool(name="pconv", bufs=3, space="PSUM"))
        plstm = ctx.enter_context(tc.tile_pool(name="plstm", bufs=1, space="PSUM"))
        psml = ctx.enter_context(tc.tile_pool(name="psml", bufs=2, space="PSUM"))

        # ---- DRAM scratch ----
        stats_loc = dram.tile([24, BHW], f32)
        stats_sh = dram.tile([24, BHW], f32, addr_space="Shared")
        adram = dram.tile([12, BHW], mdt)
        md_loc = [dram.tile([P, PADH], mdt, name=f"md_loc{h}") for h in range(2)]
        md_sh = [dram.tile([NCORES, P, PADH], mdt, addr_space="Shared",
                           name=f"md_sh{h}") for h in range(2)]
        y1_loc = [dram.tile([P, PADH], mdt, name=f"y1_loc{h}") for h in range(2)]
        y1_sh = [dram.tile([NCORES, P, PADH], mdt, addr_space="Shared",
                           name=f"y1_sh{h}") for h in range(2)]
        xa_loc = [dram.tile([P, PADH], mdt, name=f"xa_loc{h}") for h in range(2)]
        xa_sh = [dram.tile([NCORES, P, PADH], mdt, addr_space="Shared",
                           name=f"xa_sh{h}") for h in range(2)]

        # ---- warmup collective: pulls the ncfw init barrier to t~0 so it
        #      overlaps the attention prologue ----
        warm_loc = dram.tile([1, 8], f32)
        warm_sh = dram.tile([1, 8], f32, addr_space="Shared")
        warm_s = small.tile([1, 8], f32, name="warm_s")
        nc.vector.memset(warm_s[:], 0.0)
        nc.gpsimd.dma_start(warm_loc[:], warm_s[:])
        nc.gpsimd.collective_compute(
            "AllGather", mybir.AluOpType.bypass, replica_groups=RG,
            ins=[warm_loc[0:1, 0:1].opt()], outs=[warm_sh[:].opt()])

        # ---- constants / small loads ----
        b1_s = const.tile([P, 1], f32)
        nc.sync.dma_start(b1_s[:], b1[:])
        b2_s = const.tile([P, 1], f32)
        nc.sync.dma_start(b2_s[:], b2[:])
        b4_s = const.tile([P, NG], f32)
        nc.sync.dma_start(b4_s[:], b4[:])
        fct_s = const.tile([P, 7], f32)
        nc.sync.dma_start(fct_s[:], fct[:])
        fcb_s = const.tile([7, 1], f32)
        nc.sync.dma_start(fcb_s[:], fcb[:])
        # one-hot column bank: Z[:, 23] = 1, else 0.  view(i) = Z[:, 23-i:47-i]
        # is a [P, 24] matrix whose column i is all-ones.
        Z = const.tile([P, 47], f32)
        nc.vector.memset(Z[:], 0.0)
        nc.vector.memset(Z[:, 23:24], 1.0)

        def ones_lhsT(i, m=24, kdim=P):
            return Z[0:kdim, 23 - i:23 - i + m]

        # ---- attention inputs + encoder half of conv1 input ----
        mem_s = att.tile([P, N, B, HW], f32)
        nc.sync.dma_start(mem_s[:], mem[:])
        oap_s = att.tile([P, B, HW], f32)
        nc.sync.dma_start(oap_s[:], oap[:])
        htr_s = att.tile([P, B, HW], f32)
        nc.sync.dma_start(htr_s[:], htr[:])
        x4e = xpool.tile([P, CT2, B, Hp, Wp], mdt, name="x4e")
        nc.sync.dma_start(x4e[:].rearrange("p t b h w -> p t (b h w)"), xenc[:])

        # ---- local attention stats (partial over this core's 128 channels) --
        # stats-AllReduce-critical products first (PE consumes them asap)
        prA = att.tile([P, N, B, HW], f32, tag="bigA")
        nc.vector.tensor_mul(prA[:], mem_s[:],
                             oap_s[:, None].broadcast_to([P, N, B, HW]))
        mem2 = att.tile([P, N, B, HW], f32)
        nc.vector.tensor_mul(mem2[:], mem_s[:], mem_s[:])
        oap2 = prods.tile([P, B, HW], f32, name="oap2", tag="pb")
        nc.vector.tensor_mul(oap2[:], oap_s[:], oap_s[:])
        prB = att.tile([P, N, B, HW], f32, tag="bigB")
        nc.vector.tensor_mul(prB[:], mem_s[:],
                             htr_s[:, None].broadcast_to([P, N, B, HW]))
        numB = att.tile([P, N, B], f32)
        nc.vector.reduce_sum(numB[:], prB[:], axis=AX.X)
        htr2 = att.tile([P, B, HW], f32)
        nc.vector.tensor_mul(htr2[:], htr_s[:], htr_s[:])
        htr2s = att.tile([P, B], f32)
        nc.vector.reduce_sum(htr2s[:], htr2[:], axis=AX.X)
        mem2s = att.tile([P, N, B], f32)
        nc.vector.reduce_sum(mem2s[:], mem2[:], axis=AX.X)
        sqm = att.tile([P, N, B], f32)
        nc.scalar.activation(sqm[:], mem2s[:], AF.Sqrt)
        sqh = att.tile([P, B], f32)
        nc.scalar.activation(sqh[:], htr2s[:], AF.Sqrt)
        denb = att.tile([P, N, B], f32)
        nc.vector.tensor_mul(denb[:], sqm[:],
                             sqh[:, None, :].broadcast_to([P, N, B]))
        nc.vector.tensor_scalar_max(denb[:], denb[:], EPS)
        rdb = att.tile([P, N, B], f32)
        nc.vector.reciprocal_approx_fast(rdb[:], denb[:])
        csb = att.tile([P, N, B], f32)
        nc.vector.tensor_mul(csb[:], numB[:], rdb[:])

        # partition-sum stats into PSUM via one-hot matmuls:
        # rows 0..10 num_a[n]; 11..21 |mem_n|^2; 22 |oap|^2; 23 zero
        stats_ps = psml.tile([24, BHW], f32, name="stats_ps", tag="sm")
        nmm = 2 * N + 1
        k = 0
        for n in range(N):
            nc.tensor.matmul(stats_ps[:], lhsT=ones_lhsT(n),
                             rhs=prA[:, n].rearrange("p b x -> p (b x)"),
                             start=(k == 0), stop=(k == nmm - 1))
            k += 1
        for n in range(N):
            nc.tensor.matmul(stats_ps[:], lhsT=ones_lhsT(11 + n),
                             rhs=mem2[:, n].rearrange("p b x -> p (b x)"),
                             start=False, stop=(k == nmm - 1))
            k += 1
        nc.tensor.matmul(stats_ps[:], lhsT=ones_lhsT(22),
                         rhs=oap2[:].rearrange("p b x -> p (b x)"),
                         start=False, stop=True)
        csb_ps = psml.tile([1, N * B], f32, name="csb_ps", tag="sm")
        nc.tensor.matmul(csb_ps[:], lhsT=Z[:, 23:24],
                         rhs=csb[:].rearrange("p n b -> p (n b)"))

        # pack + AllReduce
        staging = small.tile([24, BHW], f32, name="staging")
        nc.vector.tensor_copy(staging[:], stats_ps[:])
        csb_s = small.tile([1, BHW], f32, name="csb_s")
        nc.vector.memset(csb_s[:], 0.0)
        nc.vector.tensor_copy(csb_s[0:1, 0:N * B], csb_ps[:])
        nc.scalar.dma_start(stats_loc[0:23, :], staging[0:23, :])
        nc.scalar.dma_start(stats_loc[23:24, :], csb_s[:])
        nc.gpsimd.collective_compute(
            "AllReduce", mybir.AluOpType.add, replica_groups=RG,
            ins=[stats_loc[:].opt()], outs=[stats_sh[:].opt()])

        # pre-AllReduce shadow work: w_n = mem_n * csb_n in place over mem_s
        nc.vector.tensor_mul(
            mem_s[:], mem_s[:],
            csb[:, :, :, None].broadcast_to([P, N, B, HW]))

        def conv_part(ps, wdram, wsel, xtiles, trange, tag, start, stop,
                      bh=None, skip_gc=False):
            """Accumulate 3x3 conv ci-tiles over trange into psum ps.
            bh: None = all batches; 0/1 = batch half."""
            tlist = list(trange)
            first, last = True, False
            for c0 in range(0, len(tlist), WCHUNK):
                chunk = tlist[c0:c0 + WCHUNK]
                cn = len(chunk)
                wc = wpool.tile([P, cn, 9, P], mdt, name=f"wc_{tag}_{c0}",
                                tag="w")
                nc.sync.dma_start(wc[:], wsel(wdram, chunk[0], cn))
                for ti, t in enumerate(chunk):
                    xt = xtiles(t)
                    if bh is None:
                        xv = xt[:, :, :, :]
                    else:
                        xv = xt[:, bh * HB:(bh + 1) * HB]
                    for j in range(9):
                        dy, dx = j // 3, j % 3
                        last = (c0 + ti == len(tlist) - 1) and (j == 8)
                        nc.tensor.matmul(
                            ps[:], lhsT=wc[:, ti, j, :],
                            rhs=xv[:, :, dy:dy + H, dx:dx + W],
                            start=(start and first), stop=(stop and last),
                            skip_group_check=skip_gc)
                        first = False

        # ---- conv1 encoder half (full batch): runs during barrier + AR.
        #      The group is left open (stop=False); the memory-half matmuls
        #      accumulate into per-batch-half subregions of the same tile ----
        y1p = pconv.tile([P, B, H, W], f32, name="y1p", tag="cv")
        conv_part(y1p, w1, lambda wd, t0, cn: wd[:, t0:t0 + cn],
                  lambda t: x4e[:, t], range(CT2), "w1a", True, False)

        # ---- post-AllReduce: g_n = e_n * rcb_n, rs ----
        na_g = small.tile([N, BHW], f32, name="na_g")
        nc.scalar.dma_start(na_g[:], stats_sh[0:N, :])
        ms_g = small.tile([N, BHW], f32, name="ms_g")
        nc.sync.dma_start(ms_g[:], stats_sh[N:2 * N, :])
        ob_g = small.tile([N, BHW], f32, name="ob_g")
        nc.scalar.dma_start(ob_g[:], stats_sh[22, :][None].broadcast_to([N, BHW]))
        cbs = small.tile([N, B], f32, name="cbs")
        nc.sync.dma_start(cbs[:], stats_sh[23, 0:N * B].rearrange(
            "(n b) -> n b", n=N))
        sqa = small.tile([N, BHW], f32, name="sqa")
        nc.scalar.activation(sqa[:], ms_g[:], AF.Sqrt)
        sqo = small.tile([N, BHW], f32, name="sqo")
        nc.scalar.activation(sqo[:], ob_g[:], AF.Sqrt)
        dena = small.tile([N, BHW], f32, name="dena")
        nc.vector.tensor_mul(dena[:], sqa[:], sqo[:])
        nc.vector.tensor_scalar_max(dena[:], dena[:], EPS)
        rda = small.tile([N, BHW], f32, name="rda")
        nc.vector.reciprocal_approx_fast(rda[:], dena[:])
        estage = small.tile([N, BHW], f32, name="estage")
        csa = small.tile([N, BHW], f32, name="csa")
        nc.vector.tensor_mul(csa[:], na_g[:], rda[:])
        nc.scalar.activation(estage[:], csa[:], AF.Exp)
        se_ps = psml.tile([1, BHW], f32, name="se_ps", tag="sm")
        nc.tensor.matmul(se_ps[:], lhsT=Z[0:N, 23:24], rhs=estage[:])
        rs_s = small.tile([1, BHW], f32, name="rs_s")
        sef = small.tile([1, BHW], f32, name="sef")
        nc.vector.tensor_copy(sef[:], se_ps[:])
        nc.vector.reciprocal_approx_fast(rs_s[:], sef[:])
        rcbs = small.tile([N, B], f32, name="rcbs")
        nc.vector.tensor_scalar_add(cbs[:], cbs[:], EPS)
        nc.vector.reciprocal_approx_fast(rcbs[:], cbs[:])
        gst = small.tile([N, B, HW], mdt, name="gst")
        nc.vector.tensor_mul(gst[:],
                             estage[:].rearrange("n (b x) -> n b x", b=B),
                             rcbs[:, :, None].broadcast_to([N, B, HW]))
        rs16 = small.tile([1, BHW], mdt, name="rs16")
        nc.vector.tensor_copy(rs16[:], rs_s[:])
        nc.scalar.dma_start(adram[0:N, :], gst[:].rearrange("n b x -> n (b x)"))
        nc.scalar.dma_start(adram[N:N + 1, :], rs16[:])
        ab = att.tile([P, 12, B, HW], mdt)
        nc.scalar.dma_start(ab[:].rearrange("p r b x -> p r (b x)"),
                            adram[:][None].broadcast_to([P, 12, BHW]))

        # ---- M_dash halves: macc_h = rs * sum_n g_n * w_n,
        #      each half AllGathers while the other computes ----
        x4m = xpool.tile([P, CT2, B, Hp, Wp], mdt, name="x4m")
        for h in range(2):
            bs = slice(h * HB, (h + 1) * HB)
            gw = att.tile([P, N, HB, HW], f32, name=f"gw{h}", tag="bigB")
            nc.vector.tensor_mul(gw[:], mem_s[:, :, bs], ab[:, 0:N, bs])
            macc = prods.tile([P, HB, HW], f32, name=f"macc{h}", tag="pb")
            nc.vector.reduce_sum(
                macc[:], gw[:].rearrange("p n b x -> p b x n"), axis=AX.X)
            mpad = xpool.tile([P, HB, Hp, Wp], mdt, name=f"mpad{h}")
            nc.vector.memset(mpad[:], 0.0)
            for b in range(HB):
                nc.vector.tensor_mul(
                    mpad[:, b, 1:1 + H, 1:1 + W],
                    macc[:, b].rearrange("p (h w) -> p h w", h=H),
                    ab[:, N, h * HB + b].rearrange("p (h w) -> p h w", h=H))
            nc.scalar.dma_start(md_loc[h][:],
                                mpad[:].rearrange("p b h w -> p (b h w)"))
            nc.gpsimd.collective_compute(
                "AllGather", mybir.AluOpType.bypass, replica_groups=RG,
                ins=[md_loc[h][:].opt()], outs=[md_sh[h][:].opt()])
            for t in range(CT2):
                nc.gpsimd.dma_start(
                    x4m[:, t, bs].rearrange("p b h w -> p (b h w)"),
                    md_sh[h][t])

        # ---- conv1 memory half, per batch half; epilogue + y1 AllGather ----
        x4y = xpool.tile([P, CT2, B, Hp, Wp], mdt, name="x4y")
        for h in range(2):
            bs = slice(h * HB, (h + 1) * HB)
            yv = y1p[:, bs]
            conv_part(yv, w1, lambda wd, t0, cn: wd[:, t0:t0 + cn],
                      lambda t: x4m[:, t - CT2], range(CT2, CT1), f"w1b{h}",
                      False, True, bh=h, skip_gc=(h == 1))
            yb = small.tile([P, HB, H, W], f32, name=f"yb{h}")
            nc.vector.tensor_scalar_add(yb[:], yv, b1_s[:])
            y1pad = xpool.tile([P, HB, Hp, Wp], mdt, name=f"y1pad{h}")
            nc.vector.memset(y1pad[:], 0.0)
            for b in range(HB):
                nc.vector.scalar_tensor_tensor(
                    y1pad[:, b, 1:1 + H, 1:1 + W], yb[:, b], 0.1, yb[:, b],
                    op0=ALU.mult, op1=ALU.max)
            nc.scalar.dma_start(y1_loc[h][:],
                                y1pad[:].rearrange("p b h w -> p (b h w)"))
            nc.gpsimd.collective_compute(
                "AllGather", mybir.AluOpType.bypass, replica_groups=RG,
                ins=[y1_loc[h][:].opt()], outs=[y1_sh[h][:].opt()])
            for t in range(CT2):
                nc.gpsimd.dma_start(
                    x4y[:, t, bs].rearrange("p b h w -> p (b h w)"),
                    y1_sh[h][t])

        # ---- conv2 per batch half; epilogue + xA AllGather ----
        x4x = xpool.tile([P, CT2, B, Hp, Wp], mdt, name="x4x")
        for h in range(2):
            bs = slice(h * HB, (h + 1) * HB)
            xq = pconv.tile([P, HB, H, W], f32, name=f"xq{h}", tag="cv")
            conv_part(xq, w2, lambda wd, t0, cn: wd[:, t0:t0 + cn],
                      lambda t: x4y[:, t], range(CT2), f"w2{h}",
                      True, True, bh=h)
            xb = small.tile([P, HB, H, W], f32, name=f"xb{h}")
            nc.vector.tensor_scalar_add(xb[:], xq[:], b2_s[:])
            xapad = xpool.tile([P, HB, Hp, Wp], mdt, name=f"xapad{h}")
            nc.vector.memset(xapad[:], 0.0)
            for b in range(HB):
                nc.vector.scalar_tensor_tensor(
                    xapad[:, b, 1:1 + H, 1:1 + W], xb[:, b], 0.1, xb[:, b],
                    op0=ALU.mult, op1=ALU.max)
            nc.scalar.dma_start(xa_loc[h][:],
                                xapad[:].rearrange("p b h w -> p (b h w)"))
            nc.gpsimd.collective_compute(
                "AllGather", mybir.AluOpType.bypass, replica_groups=RG,
                ins=[xa_loc[h][:].opt()], outs=[xa_sh[h][:].opt()])
            for t in range(CT2):
                nc.gpsimd.dma_start(
                    x4x[:, t, bs].rearrange("p b h w -> p (b h w)"),
                    xa_sh[h][t])

        # ---- ConvLSTM gates, full batch.  Order i, g, o so the c-path
        #      nonlinearities overlap the o-gate matmuls ----
        gps = {}
        for g in (0, 2, 1):
            gp = plstm.tile([P, B, H, W], f32, name=f"gate{g}", tag=f"g{g}")
            conv_part(gp, w4, lambda wd, t0, cn, g=g: wd[:, g, t0:t0 + cn],
                      lambda t: x4x[:, t], range(CT2), f"w4g{g}", True, True)
            gps[g] = gp
            if g == 0:
                si = small.tile([P, B, H, W], f32, name="si")
                nc.scalar.activation(si[:], gp[:], AF.Sigmoid,
                                     bias=b4_s[:, 0:1])
            elif g == 2:
                tg = small.tile([P, B, H, W], f32, name="tg")
                nc.scalar.activation(tg[:], gp[:], AF.Tanh, bias=b4_s[:, 2:3])
                cc = small.tile([P, B, H, W], f32, name="cc")
                nc.vector.tensor_mul(cc[:], si[:], tg[:])
                th = small.tile([P, B, H, W], f32, name="th")
                nc.scalar.activation(th[:], cc[:], AF.Tanh)
        so = small.tile([P, B, H, W], f32, name="so")
        nc.scalar.activation(so[:], gps[1][:], AF.Sigmoid, bias=b4_s[:, 1:2])
        hh = small.tile([P, B, H, W], f32, name="hh")
        nc.vector.tensor_mul(hh[:], so[:], th[:])

        # ---- gap + fc (1/HW folded into fct; b_fc/NCORES folded into fcb) ----
        gap = small.tile([P, B], f32, name="gap")
        nc.vector.reduce_sum(gap[:], hh[:], axis=AX.XY)
        pose_ps = psml.tile([7, B], f32, name="pose_ps", tag="sm")
        nc.tensor.matmul(pose_ps[:], lhsT=fct_s[:], rhs=gap[:])
        posec = small.tile([7, B], f32, name="posec")
        nc.scalar.activation(posec[:], pose_ps[:], AF.Identity, bias=fcb_s[:])
        # per-core partial pose (b_fc/NCORES folded in); host gather sums the
        # 8 sum-shards into the full output
        nc.scalar.dma_start(pose[:], posec[:])

    nc.compile()
    return nc


# ---------------------------------------------------------------- host prep
def prep_core_inputs(inputs, mm_dt_name=MM_DT):
    """Build the 8 per-core input maps from the full problem inputs."""
    mm_np = np.float16 if mm_dt_name == "float16" else np.float32
    f32 = np.float32

    memory = np.asarray(inputs["memory"], f32)      # [N,B,C,H,W]
    out_enc = np.asarray(inputs["out_enc"], f32)    # [B,C,H,W]
    h_track = np.asarray(inputs["h_track"], f32)
    outA_prev = np.asarray(inputs["outA_prev"], f32)
    w_conv1 = np.asarray(inputs["w_conv1"], f32)    # [C,2C,3,3]
    b_conv1 = np.asarray(inputs["b_conv1"], f32)
    w_conv2 = np.asarray(inputs["w_conv2"], f32)    # [C,C,3,3]
    b_conv2 = np.asarray(inputs["b_conv2"], f32)
    w_lstm = np.asarray(inputs["w_lstm"], f32)      # [4C,2C,3,3]
    b_lstm = np.asarray(inputs["b_lstm"], f32)
    w_fc = np.asarray(inputs["w_fc"], f32)          # [7,C]
    b_fc = np.asarray(inputs["b_fc"], f32)

    # shared padded encoder activations: [P, CT2, B*Hp*Wp]
    xe = np.zeros((C, B, Hp, Wp), f32)
    xe[:, :, 1:1 + H, 1:1 + W] = out_enc.transpose(1, 0, 2, 3)
    xenc = np.ascontiguousarray(
        xe.reshape(CT2, P, B * Hp * Wp).transpose(1, 0, 2)).astype(mm_np)

    def wt(wslice, nt):
        # [128co, nt*128ci, 3, 3] -> [128ci, nt, 9, 128co]
        a = wslice.reshape(P, nt, P, 9)
        return np.ascontiguousarray(a.transpose(2, 1, 3, 0)).astype(mm_np)

    gbase = [0, 2 * C, 3 * C]   # i, o, g rows in w_lstm / b_lstm

    maps = []
    for k in range(NCORES):
        ck = slice(k * P, (k + 1) * P)
        m = {}
        m["mem"] = np.ascontiguousarray(
            memory[:, :, ck].transpose(2, 0, 1, 3, 4).reshape(P, N, B, HW))
        m["oap"] = np.ascontiguousarray(
            outA_prev[:, ck].transpose(1, 0, 2, 3).reshape(P, B, HW))
        m["htr"] = np.ascontiguousarray(
            h_track[:, ck].transpose(1, 0, 2, 3).reshape(P, B, HW))
        m["xenc"] = xenc
        m["w1"] = wt(w_conv1[ck].reshape(P, 2 * C, 9), CT1)
        m["w2"] = wt(w_conv2[ck].reshape(P, C, 9), CT2)
        m["w4"] = np.ascontiguousarray(np.stack(
            [wt(w_lstm[g + k * P:g + (k + 1) * P, :C].reshape(P, C, 9), CT2)
             for g in gbase], axis=1))
        m["b1"] = b_conv1[ck].reshape(P, 1).copy()
        m["b2"] = b_conv2[ck].reshape(P, 1).copy()
        m["b4"] = np.ascontiguousarray(
            np.stack([b_lstm[g + k * P:g + (k + 1) * P] for g in gbase], axis=1))
        m["fct"] = np.ascontiguousarray(w_fc[:, ck].T) / float(HW)
        m["fcb"] = (b_fc / float(NCORES)).reshape(7, 1).copy()
        maps.append(m)
    return maps


# ---------------------------------------------------------------- entry
def run(inputs, trace=False, mm_dt_name=MM_DT):
    from concourse.bass_utils import run_bass_kernel_spmd

    key = ("prog", mm_dt_name)
    if key not in _cache:
        _cache[key] = build_program(mm_dt_name)
    nc = _cache[key]
    in_maps = prep_core_inputs(inputs, mm_dt_name)
    res = run_bass_kernel_spmd(nc, in_maps, list(range(NCORES)), trace=trace)
    acc = np.zeros((7, B), np.float64)
    for k in range(NCORES):
        acc += np.asarray(res.results[k]["pose"], np.float32)
    out = acc.T.astype(np.float32)  # [B, 7]
    return out, res


def kernel(**inputs) -> np.ndarray:
    out, _ = run(inputs, trace=False)
    return out
